# revision 53
# baseline (speedup 1.0000x reference)
"""Mamba SSM block on 8 TRN2 NeuronCores (Bass/Tile, SPMD).

Sharding: d_inner (2048 -> 256/core) for in_proj, conv, dt/B/C projections and
the selective scan; the output projection is token-sharded. Collectives:
  - AllReduce of x_dbl projection partials [96, 512] bf16 per 512-token chunk
  - Four AllToAlls of the gated scan output yg (token groups 2048/1024/512/512)
    that reshard yg from d_inner-sharded to token-sharded; each core then runs
    the full (host-folded) W_c = W_out @ W_out_ssm projection for its tokens.

Scan: h[t] = exp(A dt[t]) h[t-1] + (dt[t] x[t]) B[t] via 16 tensor_tensor_scan
ops (one per state index) along tokens, chained across chunks through
per-partition `initial` APs. B/C are broadcast to 128 partitions by one
stride-0-partition DMA each per chunk (no PE/ACT involvement). The depthwise
conv runs on the PE as 4 accumulating diagonal matmuls; silu is native on ACT;
softplus is Exp+Ln so the whole mid phase shares one activation table (see
_narrow_act_tables). The h*C s-fold and the D-skip run on the PE: 16
identity-diagonal matmuls + one diag(D) matmul accumulate y = sum_s h*C +
D*xs into a PSUM bank (f32), freeing ~80us of DVE time vs DVE tree-folds.
GpSimd is avoided entirely for compute: any op running there stalls
concurrent DVE ops ~15x (measured), even on disjoint tiles. W_c stays
resident in SBUF (32KB/partition) so the tail out-projections never wait on
weight DMAs; the last chunk's A2A is split in two so the final A2A+tail
drain is short. Mid/tail weight loads are deferred past front(0) to keep
the first AllReduce's input path clear. Matmul operands and scan tensors
are bf16 (rel_err ~5e-3 vs fp32 reference); conv accumulation, PSUM folds
and the AllReduce stay fp32.
"""
import numpy as np
import ml_dtypes

import concourse.bass as bass
import concourse.tile as tile
from concourse import bacc, mybir
from concourse.bass_utils import run_bass_kernel_spmd

BFnp = ml_dtypes.bfloat16
F32 = mybir.dt.float32
BF16 = mybir.dt.bfloat16
AF = mybir.ActivationFunctionType
OP = mybir.AluOpType

NC = 8
B, L, DM = 2, 2048, 1024
DI, S, R, KC = 2048, 16, 64, 4
DIL = DI // NC            # 256 d_inner per core
NT = B * L                # 4096 tokens (batch-major)
TC = 512                  # tokens per chunk
NCH = NT // TC            # 8 chunks
NI = DIL // 128           # 2 partition tiles of local d_inner
EO = 8                    # 1024 output rows = 8 tiles of 128

# AllToAll groups: (token_start, token_count, tokens-per-block); the last
# chunk is split in two so the final A2A+tail drain is half as long
A2A_GROUPS = [(0, 2048, 256), (2048, 1024, 128), (3072, 512, 64),
              (3584, 256, 32), (3840, 256, 32)]

_NC_CACHE = {}


def _narrow_act_tables(arch):
    """Keep Exp and Ln only in natural_log_exp_and_others so the compiler's
    table-load pass puts the whole dt/a_t sequence on one activation table
    (it otherwise thrashes between exp_and_others and natural_log, ~1.3us
    per reload). Only removes entries, so every emitted load is still valid."""
    from concourse.hw_specs import get_activation_tables
    tabs = get_activation_tables(arch)  # functools.cache -> shared dict
    for name, fns in tabs.items():
        if name == "natural_log_exp_and_others":
            continue
        fns.discard(AF.Exp)
        fns.discard(AF.Ln)


def build(use_silu=True):
    """use_silu=False replaces native Silu with Sigmoid+mult (CoreSim lacks Silu)."""
    key = ("nc", use_silu)
    if key in _NC_CACHE:
        return _NC_CACHE[key]
    nc = bacc.Bacc("TRN2", target_bir_lowering=False, debug=False, num_devices=NC)
    _NC_CACHE[key] = None  # placeholder
    _narrow_act_tables(nc.m.arch)

    # ---- per-core DRAM inputs (host pre-sharded / transposed / casted) ----
    x_t = nc.dram_tensor("x_t", [DM, NT], BF16, kind="ExternalInput")         # replicated
    w_in_x = nc.dram_tensor("w_in_x", [DM, DIL], BF16, kind="ExternalInput")  # W_in[dk,:].T
    w_in_z = nc.dram_tensor("w_in_z", [DM, DIL], BF16, kind="ExternalInput")
    conv_wd = nc.dram_tensor("conv_wd", [128, NI * KC * 128], BF16, kind="ExternalInput")  # diag tap blocks
    conv_b = nc.dram_tensor("conv_b", [DIL, 1], F32, kind="ExternalInput")
    w_xp = nc.dram_tensor("w_xp", [DIL, R + 2 * S], BF16, kind="ExternalInput")  # W_xp[:,dk].T
    w_dt = nc.dram_tensor("w_dt", [R, DIL], BF16, kind="ExternalInput")          # W_dt[dk,:].T
    b_dt = nc.dram_tensor("b_dt", [DIL, 1], F32, kind="ExternalInput")
    a_mat = nc.dram_tensor("a_mat", [DIL, S], F32, kind="ExternalInput")        # -exp(A_log[dk])
    d_diag = nc.dram_tensor("d_diag", [128, NI * 128], BF16, kind="ExternalInput")  # diag(D) blocks
    w_c = nc.dram_tensor("w_c", [128, 16 * EO * 128], BF16, kind="ExternalInput")  # W_c blocks, replicated
    b_o = nc.dram_tensor("b_o", [128, EO], F32, kind="ExternalInput")
    ident = nc.dram_tensor("ident", [128, 128], BF16, kind="ExternalInput")  # PE fold identity
    out = nc.dram_tensor("out", [8 * 128, TC], F32, kind="ExternalOutput")     # [1024, my 512 tokens]

    with tile.TileContext(nc) as tc:
        with (
            tc.tile_pool(name="wpool", bufs=1) as wp,     # persistent weights
            tc.tile_pool(name="xckp", bufs=1) as xkp,     # streamed x
            tc.tile_pool(name="ygsp", bufs=1) as ygp,     # A2A output gather
            tc.tile_pool(name="work", bufs=1) as wk,      # DVE-only transients
            tc.tile_pool(name="worka", bufs=2) as wka,    # ACT/DMA-written tiles
            tc.tile_pool(name="keep", bufs=5) as kp,      # xs/g (live across chunk)
            tc.tile_pool(name="scan", bufs=1) as sc,      # big bf16 scan tiles
            tc.tile_pool(name="scana", bufs=2) as sca,    # a_t (dbl buf: ACT vs scan)
            tc.tile_pool(name="bcast", bufs=1) as bcp,    # B/C broadcast tiles
            tc.tile_pool(name="state", bufs=1) as st,     # persistent hprev/xtail
            tc.tile_pool(name="psA", bufs=2, space="PSUM") as psA,
            tc.tile_pool(name="psC", bufs=1, space="PSUM") as psC,
            tc.tile_pool(name="psB1", bufs=1, space="PSUM") as psB1,
            tc.tile_pool(name="psB2", bufs=1, space="PSUM") as psB2,
            tc.tile_pool(name="psY", bufs=1, space="PSUM") as psY,
            tc.tile_pool(name="psO", bufs=1, space="PSUM") as psO,
            tc.tile_pool(name="dram", bufs=1, space="DRAM") as dr,
        ):
            # ---------- load weights ----------
            winx = wp.tile([128, 8 * NI * 128], BF16, tag="winx")
            winz = wp.tile([128, 8 * NI * 128], BF16, tag="winz")
            nc.sync.dma_start(
                winx[:].rearrange("p (k i n) -> p k i n", k=8, i=NI),
                w_in_x.ap().rearrange("(k p) (i n) -> p k i n", p=128, i=NI))
            nc.sync.dma_start(
                winz[:].rearrange("p (k i n) -> p k i n", k=8, i=NI),
                w_in_z.ap().rearrange("(k p) (i n) -> p k i n", p=128, i=NI))
            def load_xck(c):
                t0 = c * TC
                xck = xkp.tile([128, 8 * TC], BF16, tag="xck")
                nc.sync.dma_start(
                    xck[:].rearrange("p (k t) -> p k t", k=8),
                    x_t.ap()[:, t0:t0 + TC].rearrange("(k p) t -> p k t", p=128))
                return xck

            xck0 = load_xck(0)  # chunk-0 tokens ahead of conv/xp weights
            cwd = wp.tile([128, NI * KC * 128], BF16, tag="cwd")
            nc.sync.dma_start(cwd[:], conv_wd[:, :])
            cb = wp.tile([128, NI], F32, tag="cb")
            for i in range(NI):
                nc.sync.dma_start(cb[:, i:i + 1], conv_b[i * 128:(i + 1) * 128, :])
            wxp = wp.tile([128, NI * (R + 2 * S)], BF16, tag="wxp")
            nc.sync.dma_start(
                wxp[:].rearrange("p (i n) -> p i n", i=NI),
                w_xp.ap().rearrange("(i p) n -> p i n", p=128))
            # mid/tail-phase weights: DMAs deferred to iteration c==1 so the
            # front(0) xd/AllReduce path isn't stuck behind them in the queues
            wdt = wp.tile([R, NI * 128], BF16, tag="wdt")
            bdt = wp.tile([128, NI], F32, tag="bdt")
            ddg = wp.tile([128, NI * 128], BF16, tag="ddg")
            am = wp.tile([128, NI * S], F32, tag="am")
            bo = wp.tile([128, EO], F32, tag="bo")
            idm = wp.tile([128, 128], BF16, tag="idm")

            def load_mid_weights():
                nc.sync.dma_start(wdt[:], w_dt[:, :])
                nc.sync.dma_start(ddg[:], d_diag[:, :])
                for i in range(NI):
                    sl = slice(i * 128, (i + 1) * 128)
                    nc.sync.dma_start(bdt[:, i:i + 1], b_dt[sl, :])
                    nc.sync.dma_start(am[:, i * S:(i + 1) * S], a_mat[sl, :])
                nc.sync.dma_start(bo[:], b_o[:, :])
                nc.sync.dma_start(idm[:], ident[:, :])

            # resident out-projection weights (32KB/partition): loaded once
            # mid-stream so tails never wait on W_c DMAs
            wcsb = wp.tile([128, 16 * EO * 128], BF16, tag="wcsb")

            def load_wc():
                nc.sync.dma_start(wcsb[:], w_c[:, :])

            hprev = st.tile([128, NI * S], BF16, tag="hprev")
            xtail = st.tile([128, NI * 3], BF16, tag="xtail")

            # A2A input/output DRAM tiles (local, not Shared)
            a2a_in = [dr.tile([2048, tb], BF16, tag=f"a2ain{g}", name=f"a2ain{g}")
                      for g, (_, _, tb) in enumerate(A2A_GROUPS)]
            a2a_out = [dr.tile([2048, tb], BF16, tag=f"a2aout{g}", name=f"a2aout{g}")
                       for g, (_, _, tb) in enumerate(A2A_GROUPS)]

            ctx = {}

            def front(c, xck=None):
                """x stream, in_proj, conv, silu, z-gate, x_dbl partial, AR issue."""
                t0 = c * TC
                reset = (c % (NCH // B) == 0)
                if xck is None:
                    xck = load_xck(c)

                xs_i, g_i = [], []
                for i in range(NI):
                    ps_x = psA.tile([128, TC], F32, tag="psx", bufs=1)
                    ps_z = psA.tile([128, TC], F32, tag="psz")
                    for kt in range(8):
                        wsl = slice((kt * NI + i) * 128, (kt * NI + i + 1) * 128)
                        nc.tensor.matmul(ps_x[:], winx[:, wsl], xck[:, kt * TC:(kt + 1) * TC],
                                         start=(kt == 0), stop=(kt == 7))
                    for kt in range(8):
                        wsl = slice((kt * NI + i) * 128, (kt * NI + i + 1) * 128)
                        nc.tensor.matmul(ps_z[:], winz[:, wsl], xck[:, kt * TC:(kt + 1) * TC],
                                         start=(kt == 0), stop=(kt == 7))

                    # causal depthwise conv as 4 accumulating diag matmuls on PE;
                    # xin carries a 3-token halo from the previous chunk
                    xin = wka.tile([128, TC + 3], BF16, tag="xin")
                    # halo write first: avoids a false tile-granular wait on
                    # the big copy for the gpsimd memset path
                    if reset:
                        nc.gpsimd.memset(xin[:, 0:3], 0.0)
                    else:
                        nc.vector.tensor_copy(xin[:, 0:3], xtail[:, i * 3:i * 3 + 3])
                    nc.scalar.copy(xin[:, 3:], ps_x[:])
                    nc.vector.tensor_copy(xtail[:, i * 3:i * 3 + 3], ps_x[:, TC - 3:TC])
                    ps_c = psC.tile([128, TC], F32, tag="psc")
                    for k in range(KC):
                        nc.tensor.matmul(ps_c[:], cwd[:, (i * KC + k) * 128:(i * KC + k + 1) * 128],
                                         xin[:, k:k + TC], start=(k == 0), stop=(k == KC - 1))
                    xs = kp.tile([128, TC], BF16, tag="xs")
                    g = kp.tile([128, TC], BF16, tag="g")
                    if use_silu:
                        nc.scalar.activation(xs[:], ps_c[:], AF.Silu, bias=cb[:, i:i + 1])
                        # z gate: g = silu(z) straight from PSUM
                        nc.scalar.activation(g[:], ps_z[:], AF.Silu)
                    else:
                        u = wk.tile([128, TC], F32, tag="accA")
                        nc.scalar.activation(u[:], ps_c[:], AF.Identity, bias=cb[:, i:i + 1])
                        sgu = wka.tile([128, TC], F32, tag="sgu")
                        nc.scalar.activation(sgu[:], u[:], AF.Sigmoid)
                        nc.vector.tensor_tensor(out=xs[:], in0=u[:], in1=sgu[:], op=OP.mult)
                        sgz = wka.tile([128, TC], F32, tag="sgz")
                        nc.scalar.activation(sgz[:], ps_z[:], AF.Sigmoid)
                        nc.vector.scalar_tensor_tensor(
                            out=g[:], in0=sgz[:], scalar=1.0, in1=ps_z[:],
                            op0=OP.mult, op1=OP.mult)
                    xs_i.append(xs)
                    g_i.append(g)

                # x_dbl partial + AllReduce
                ps_xd = psB1.tile([R + 2 * S, TC], F32, tag="psxd")
                for i in range(NI):
                    nc.tensor.matmul(ps_xd[:], wxp[:, i * (R + 2 * S):(i + 1) * (R + 2 * S)],
                                     xs_i[i][:], start=(i == 0), stop=(i == NI - 1))
                xd_sb = wka.tile([R + 2 * S, TC], BF16, tag="xdsb")
                nc.scalar.copy(xd_sb[:], ps_xd[:])
                xd_part = dr.tile([R + 2 * S, TC], BF16, tag=f"xdp{c % 4}")
                nc.sync.dma_start(xd_part[:], xd_sb[:])
                xd_red = nc.dram_tensor(f"xd_red_{c}", [R + 2 * S, TC], BF16, addr_space="Shared")
                nc.gpsimd.collective_compute(
                    "AllReduce", OP.add, replica_groups=[list(range(NC))],
                    ins=[xd_part[:]], outs=[xd_red.ap()])
                ctx[c] = dict(xs_i=xs_i, g_i=g_i, xd_red=xd_red)

            def mid_head(c):
                """dtr/B/C loads and dt for both i-tiles (full chunk width)."""
                xd_red = ctx[c]["xd_red"]
                dtr = wka.tile([R, TC], BF16, tag="dtr")
                nc.sync.dma_start(dtr[:], xd_red.ap()[0:R, :])
                # B/C rows broadcast to all 128 partitions: one stride-0 DMA each
                b_bc = bcp.tile([128, S * TC], BF16, tag="bbc")
                c_bc = bcp.tile([128, S * TC], BF16, tag="cbc", bufs=2)
                nc.scalar.dma_start(
                    b_bc[:].rearrange("p (s t) -> p s t", s=S),
                    xd_red.ap()[R:R + S, :].unsqueeze(0).broadcast_to([128, S, TC]))
                nc.scalar.dma_start(
                    c_bc[:].rearrange("p (s t) -> p s t", s=S),
                    xd_red.ap()[R + S:R + 2 * S, :].unsqueeze(0).broadcast_to([128, S, TC]))
                # Exp+Ln softplus keeps the whole mid phase on the
                # natural_log_exp activation table (a_t uses Exp too)
                dt_i = []
                for i in range(NI):
                    ps_dt = psB2.tile([128, TC], F32, tag="psdt")
                    nc.tensor.matmul(ps_dt[:], wdt[:, i * 128:(i + 1) * 128], dtr[:],
                                     start=True, stop=True)
                    edt = wka.tile([128, TC], BF16, tag="edt")
                    nc.scalar.activation(edt[:], ps_dt[:], AF.Exp, bias=bdt[:, i:i + 1])
                    dt = wka.tile([128, TC], BF16, tag="dtt")
                    nc.scalar.activation(dt[:], edt[:], AF.Ln, bias=1.0)
                    dt_i.append(dt)
                ctx[c].update(b_bc=b_bc, c_bc=c_bc, dt_i=dt_i)

            def mid_body(c, col0, W):
                """dtx, decay exps, bb, scans, h*C + folds, gate, yg scatter for
                token window [c*TC+col0, c*TC+col0+W)."""
                t0 = c * TC + col0
                reset = (t0 % (NT // B) == 0)
                xs_i, g_i = ctx[c]["xs_i"], ctx[c]["g_i"]
                b_bc, c_bc, dt_i = ctx[c]["b_bc"], ctx[c]["c_bc"], ctx[c]["dt_i"]
                win = slice(col0, col0 + W)

                for g, (gt0, gnt, tb) in enumerate(A2A_GROUPS):
                    if gt0 <= t0 < gt0 + gnt:
                        grp, tb_g, rel = g, tb, t0 - gt0
                        break

                # per i-tile: dtx, decay exps, bb, scans, h*C, then immediately
                # the PE fold + gate + scatter (so hc_t recycles without a
                # cross-engine stall on the NEXT i's hc write)
                for i in range(NI):
                    dt = dt_i[i]
                    dtx = wk.tile([128, TC], BF16, tag="dtx")
                    nc.vector.tensor_tensor(out=dtx[:, 0:W], in0=dt[:, win],
                                            in1=xs_i[i][:, win], op=OP.mult)

                    # a_t in two s-halves (same ACT-ahead-of-scan pipelining,
                    # half the SBUF of a monolithic double-buffered tile)
                    SH = S // 2
                    a_h = []
                    for h in range(2):
                        ah = sca.tile([128, SH * TC], BF16, tag="a_h")
                        for s in range(SH):
                            nc.scalar.activation(
                                ah[:, s * TC + col0:s * TC + col0 + W], dt[:, win], AF.Exp,
                                scale=am[:, i * S + h * SH + s:i * S + h * SH + s + 1])
                        a_h.append(ah)

                    bb_t = sc.tile([128, S * TC], BF16, tag="bb_t")
                    bb3 = bb_t[:].rearrange("p (s t) -> p s t", s=S)
                    nc.vector.tensor_tensor(
                        out=bb3[:, :, win],
                        in0=dtx[:, 0:W].unsqueeze(1).broadcast_to([128, S, W]),
                        in1=b_bc[:].rearrange("p (s t) -> p s t", s=S)[:, :, win],
                        op=OP.mult)

                    h_t = sc.tile([128, S * TC], BF16, tag="h_t")
                    if reset:
                        nc.gpsimd.memset(hprev[:, i * S:(i + 1) * S], 0.0)
                    for s in range(S):
                        nc.vector.tensor_tensor_scan(
                            h_t[:, s * TC + col0:s * TC + col0 + W],
                            a_h[s // SH][:, (s % SH) * TC + col0:(s % SH) * TC + col0 + W],
                            bb_t[:, s * TC + col0:s * TC + col0 + W],
                            hprev[:, i * S + s:i * S + s + 1],
                            op0=OP.mult, op1=OP.add)
                    nc.vector.tensor_copy(
                        hprev[:, i * S:(i + 1) * S],
                        h_t[:].rearrange("p (s t) -> p s t", s=S)[:, :, col0 + W - 1])

                    hc_t = sc.tile([128, S * TC], BF16, tag="hc_t")
                    hc3 = hc_t[:].rearrange("p (s t) -> p s t", s=S)
                    nc.vector.tensor_tensor(
                        out=hc3[:, :, win],
                        in0=h_t[:].rearrange("p (s t) -> p s t", s=S)[:, :, win],
                        in1=c_bc[:].rearrange("p (s t) -> p s t", s=S)[:, :, win],
                        op=OP.mult)

                    # s-fold + D-skip off the DVE: 16 identity-diag matmuls and
                    # one diag(D) matmul accumulate y[d,t] = sum_s hc[d,s,t]
                    # + D[d]*xs[d,t] into one PSUM bank (f32)
                    ps_y = psY.tile([128, TC], F32, tag="psy")
                    for s in range(S):
                        nc.tensor.matmul(ps_y[:, 0:W],
                                         idm[:, :],
                                         hc_t[:, s * TC + col0:s * TC + col0 + W],
                                         start=(s == 0), stop=False)
                    nc.tensor.matmul(ps_y[:, 0:W], ddg[:, i * 128:(i + 1) * 128],
                                     xs_i[i][:, win], start=False, stop=True)
                    yg = wk.tile([128, TC], BF16, tag="yg")
                    nc.vector.tensor_tensor(out=yg[:, 0:W], in0=ps_y[:, 0:W],
                                            in1=g_i[i][:, win], op=OP.mult)
                    # scatter yg into the A2A input: row-block = token-block,
                    # rows within block = my local d channels
                    for b in range(rel // tb_g, (rel + W + tb_g - 1) // tb_g):
                        colA = max(b * tb_g - rel, 0)
                        colB = min((b + 1) * tb_g - rel, W)
                        nc.sync.dma_start(
                            a2a_in[grp][b * 256 + i * 128: b * 256 + (i + 1) * 128,
                                        rel + colA - b * tb_g: rel + colB - b * tb_g],
                            yg[:, colA:colB])

            def a2a(g):
                nc.gpsimd.collective_compute(
                    "AllToAll", OP.bypass, replica_groups=[list(range(NC))],
                    ins=[a2a_in[g][:]], outs=[a2a_out[g][:]])

            def tail(g):
                """gather a2a_out[g], out projection, bias, store my token cols."""
                tb = A2A_GROUPS[g][2]
                col0 = sum(t for _, _, t in A2A_GROUPS[:g])
                ygs = ygp.tile([128, 16 * tb], BF16, tag="ygs")
                nc.sync.dma_start(
                    ygs[:].rearrange("p (k t) -> p k t", k=16),
                    a2a_out[g][:, :].rearrange("(k p) t -> p k t", p=128))
                for e in range(EO):
                    ps_o = psO.tile([128, tb], F32, tag="pso")
                    for kt in range(16):
                        nc.tensor.matmul(ps_o[:], wcsb[:, (e * 16 + kt) * 128:(e * 16 + kt + 1) * 128],
                                         ygs[:, kt * tb:(kt + 1) * tb],
                                         start=(kt == 0), stop=(kt == 15))
                    o_sb = wka.tile([128, tb], F32, tag="osb")
                    nc.scalar.activation(o_sb[:], ps_o[:], AF.Identity, bias=bo[:, e:e + 1])
                    nc.sync.dma_start(out[e * 128:(e + 1) * 128, col0:col0 + tb], o_sb[:])

            for c in range(NCH + 3):
                if c < NCH:
                    front(c, xck0 if c == 0 else None)
                if c == 1:
                    load_mid_weights()
                elif c == 3:
                    load_wc()
                m = c - 2
                if 0 <= m < 8:
                    mid_head(m)
                    if m == 7:
                        # last chunk split in half so its two A2As + tails drain fast
                        mid_body(7, 0, TC // 2)
                        a2a(3)
                        mid_body(7, TC // 2, TC // 2)
                        a2a(4)
                    else:
                        mid_body(m, 0, TC)
                        if m == 3:
                            a2a(0)
                        elif m == 5:
                            a2a(1)
                        elif m == 6:
                            a2a(2)
                if c == 7:
                    tail(0)
                elif c == 8:
                    tail(1)
                elif c == 9:
                    tail(2)
                elif c == 10:
                    tail(3)
                    tail(4)

    nc.compile()
    _NC_CACHE[key] = nc
    return nc


def _prep_inputs(inputs):
    x = np.ascontiguousarray(np.asarray(inputs["x"], np.float32))
    W_in = np.asarray(inputs["W_in"], np.float32)
    conv_w = np.asarray(inputs["conv_w"], np.float32)
    conv_b = np.asarray(inputs["conv_b"], np.float32)
    W_xp = np.asarray(inputs["W_xp"], np.float32)
    W_dt = np.asarray(inputs["W_dt"], np.float32)
    b_dt = np.asarray(inputs["b_dt"], np.float32)
    A_log = np.asarray(inputs["A_log"], np.float32)
    D = np.asarray(inputs["D"], np.float32)
    W_out_ssm = np.asarray(inputs["W_out_ssm"], np.float32)
    W_out = np.asarray(inputs["W_out"], np.float32)
    b_out = np.asarray(inputs["b_out"], np.float32)

    A = -np.exp(A_log)
    W_c = (W_out.astype(np.float64) @ W_out_ssm.astype(np.float64)).astype(np.float32)
    x_t = np.ascontiguousarray(x.reshape(NT, DM).T.astype(BFnp))  # [DM, NT] bf16

    # conv taps as diagonal stationary blocks per (local i-tile, tap)
    def conv_diag(dsl):
        cw = conv_w[dsl, 0, :]  # [DIL, KC]
        blk = np.zeros((128, NI * KC * 128), BFnp)
        for i in range(NI):
            for k in range(KC):
                d = np.zeros((128, 128), np.float32)
                np.fill_diagonal(d, cw[i * 128:(i + 1) * 128, k])
                blk[:, (i * KC + k) * 128:(i * KC + k + 1) * 128] = d.astype(BFnp)
        return blk

    def d_diag_blocks(dsl):
        dval = D[dsl]
        blk = np.zeros((128, NI * 128), BFnp)
        for i in range(NI):
            dd = np.zeros((128, 128), np.float32)
            np.fill_diagonal(dd, dval[i * 128:(i + 1) * 128])
            blk[:, i * 128:(i + 1) * 128] = dd.astype(BFnp)
        return blk

    # W_c as stationary blocks: [128, (kt*EO+e)*128] = W_c[e-rows, kt-cols].T
    wc_blk = np.zeros((128, 16 * EO * 128), BFnp)
    for e in range(EO):
        for kt in range(16):
            blk = W_c[e * 128:(e + 1) * 128, kt * 128:(kt + 1) * 128].T
            wc_blk[:, (e * 16 + kt) * 128:(e * 16 + kt + 1) * 128] = blk.astype(BFnp)
    bo_blk = np.ascontiguousarray(b_out.reshape(EO, 128).T)  # [128, EO]

    in_maps = []
    for k in range(NC):
        dsl = slice(k * DIL, (k + 1) * DIL)
        in_maps.append({
            "x_t": x_t,
            "w_in_x": np.ascontiguousarray(W_in[dsl, :].T.astype(BFnp)),
            "w_in_z": np.ascontiguousarray(
                W_in[DI + k * DIL: DI + (k + 1) * DIL, :].T.astype(BFnp)),
            "conv_wd": conv_diag(dsl),
            "conv_b": np.ascontiguousarray(conv_b[dsl][:, None]),
            "w_xp": np.ascontiguousarray(W_xp[:, dsl].T.astype(BFnp)),
            "w_dt": np.ascontiguousarray(W_dt[dsl, :].T.astype(BFnp)),
            "b_dt": np.ascontiguousarray(b_dt[dsl][:, None]),
            "a_mat": np.ascontiguousarray(A[dsl, :]),
            "d_diag": d_diag_blocks(dsl),
            "w_c": wc_blk,
            "b_o": bo_blk,
            "ident": np.eye(128, dtype=BFnp),
        })
    return in_maps


def _assemble(results):
    full = np.zeros((DM, NT), np.float32)
    for k in range(NC):
        o = results[k]["out"]  # [1024, 512]: token cols per A2A group
        col = 0
        for tok0, ntok, tb in A2A_GROUPS:
            full[:, tok0 + tb * k: tok0 + tb * (k + 1)] = o[:, col:col + tb]
            col += tb
    return np.ascontiguousarray(full.T).reshape(B, L, DM)


def kernel(**inputs):
    nc = build()
    in_maps = _prep_inputs(inputs)
    res = run_bass_kernel_spmd(nc, in_maps, core_ids=list(range(NC)))
    return _assemble(res.results)


def kernel_sim(**inputs):
    """Run through MultiCoreSim instead of HW (for debugging)."""
    from concourse.bass_interp import MultiCoreSim
    nc = build(use_silu=False)
    in_maps = _prep_inputs(inputs)
    sim = MultiCoreSim(nc, num_cores=NC)
    for k in range(NC):
        for name, arr in in_maps[k].items():
            sim.cores[k].tensor(name)[:] = arr
    sim.simulate(check_with_hw=False)
    results = [{"out": sim.cores[k].tensor("out").copy()} for k in range(NC)]
    return _assemble(results)

